# revision 1
# baseline (speedup 1.0000x reference)
"""Trainium2 Bass kernel for nn_BatchRankingLoss (n=8192, 8 NeuronCores).

Math: reference computes sum over pairs i<j of relu(-(p_j-p_i)*sign(l_j-l_i) + 2).
The sum runs over UNORDERED pairs and is invariant to re-indexing, so we sort by
labels on the host: with q = preds[argsort(labels)], the loss becomes
    sum_{u<v} relu(2 + q_u - q_v)
(plus an exact O(#ties) host correction for tied labels, where sign()=0).

Device strategy (SPMD, 8 cores, one shared program). 64 row-tiles of 128 rows;
core k owns tiles {k+16m, 15-k+16m}, presented as 8 fixed-width "slots" of
[16,14,12,10,8,6,4,2] 512-col chunks (window starts at the diagonal block;
unused tail columns zero-padded). Three engines are saturated in parallel:

- PE route (46 chunks, slot-proportional, incl. every diagonal chunk):
  K=16 bf16 matmul per 512-col chunk -> t = q_u + (2 - q_v) in f32 PSUM (rhs
  packed into 8 partition-pair "streams", zero lhsT lanes select the stream).
  Diagonal chunks get a second [128,128] matmul adding -1e9 on the lower
  triangle. PSUM groups are reduced by either:
    ACT: activation(Relu, accum_out) -> sum relu(t)
    DVE: tensor_reduce(add, abs) -> sum |t|, combined with the analytic linear
         term sum(t) (affine in q_u, per-core inputs) via relu = (t + |t|)/2.
- ACT-direct route (26 chunks): a broadcast tile QB[128, 13312] holds
  bf16(2 - q_v) replicated across partitions; activation(Relu, bias=q_u,
  accum_out) computes sum_v relu(2 - q_v + q_u) in ONE ACT pass (no PE, no
  separate reduce). Padded columns hold -1000 so relu kills them.

Each core outputs a [128,1] partial; host sums 8x128 partials + tie correction.
"""

import numpy as np

N = 8192
NBLK = 64
SLOT_CHUNKS = [16, 14, 12, 10, 8, 6, 4, 2]    # 512-col chunks per slot
PE_CHUNKS = [10, 9, 8, 7, 5, 4, 2, 1]         # chunks on the PE route per slot
ALT_CHUNKS = [c - p for c, p in zip(SLOT_CHUNKS, PE_CHUNKS)]   # ACT-direct
N_PE = sum(PE_CHUNKS)                          # 46
N_ALT = sum(ALT_CHUNKS)                        # 26
STREAM_CAP = 6                                 # PE chunks per stream (6*512)
QB_COLS = N_ALT * 512                          # 13312
PENALTY = -1.0e9
PAD_VAL = -1000.0

# ---------------------------------------------------------------------------
# Stream packing for the PE route
# ---------------------------------------------------------------------------

def _pack_streams():
    chunk_map = {}
    variants = []
    vmap = {}
    stream = 0
    pos = 0
    for s, nch in enumerate(PE_CHUNKS):
        for c in range(nch):
            if pos == STREAM_CAP:
                stream += 1
                pos = 0
            chunk_map[(s, c)] = (stream, pos)
            if (s, stream) not in vmap:
                vmap[(s, stream)] = len(variants)
                variants.append((s, stream))
            pos += 1
    assert stream <= 7, (stream, pos)
    return chunk_map, variants, vmap

CHUNK_MAP, VARIANTS, VMAP = _pack_streams()
NVAR = len(VARIANTS)

# ALT segment offsets in QB (per slot), in columns
ALT_OFFS = []
_o = 0
for _c in ALT_CHUNKS:
    ALT_OFFS.append(_o)
    _o += _c * 512
assert _o == QB_COLS

# ---------------------------------------------------------------------------
# Schedule: PE-route reduce groups + engine assignment
# ---------------------------------------------------------------------------

def make_schedule():
    """PE-route groups: (slot, chunk0, nchunks, is_diag, engine)."""
    groups = []
    for s, nch in enumerate(PE_CHUNKS):
        c = 0
        while c < nch:
            if c == 0 and s < 4:
                g = 1            # narrow diag group: shifts reduce work to DVE
            else:
                g = min(2, nch - c)
            groups.append([s, c, g, c == 0])
            c += g
    # diag groups forced to ACT; others balance DVE-heavy (ACT also runs the
    # ACT-direct route, so give DVE everything it can take)
    act_cost = sum(ALT_CHUNKS) * 512 * 0.8333 + 8 * 370.0   # ACT-direct load
    dve_cost = 0.0
    sched = []
    for s, c0, g, diag in groups:
        w = g * 512
        ca = w * 0.8333 + 290.0
        cd = w * 1.0417 + 170.0
        if diag:
            eng = "A"
        else:
            eng = "A" if act_cost + ca <= dve_cost + cd else "D"
        if eng == "A":
            act_cost += ca
        else:
            dve_cost += cd
        sched.append((s, c0, g, diag, eng))
    return sched

SCHEDULE = make_schedule()

# ---------------------------------------------------------------------------
# Device program
# ---------------------------------------------------------------------------

_CACHE = {}

def build_program():
    import concourse.bacc as bacc
    import concourse.mybir as mybir
    from concourse.tile import TileContext

    F32 = mybir.dt.float32
    BF16 = mybir.dt.bfloat16
    AX = mybir.AxisListType
    OP = mybir.AluOpType
    AF = mybir.ActivationFunctionType

    nA = sum(1 for g in SCHEDULE if g[4] == "A") + 8   # + 8 ACT-direct groups
    nD = sum(1 for g in SCHEDULE if g[4] == "D")

    nc = bacc.Bacc(trn_type="TRN2")
    rhs_d = nc.dram_tensor("rhs", [16, STREAM_CAP * 512], BF16, kind="ExternalInput")
    lhs_d = nc.dram_tensor("lhs", [16, NVAR * 128], BF16, kind="ExternalInput")
    tri_d = nc.dram_tensor("tri", [128, 128], BF16, kind="ExternalInput")
    pen_d = nc.dram_tensor("pen", [128, 128], BF16, kind="ExternalInput")
    qb_d = nc.dram_tensor("qb", [128, QB_COLS], BF16, kind="ExternalInput")
    qcol_d = nc.dram_tensor("qcol", [128, 8], F32, kind="ExternalInput")
    lin_d = nc.dram_tensor("linab", [128, 16], F32, kind="ExternalInput")
    out_d = nc.dram_tensor("out", [128, 1], F32, kind="ExternalOutput")

    with TileContext(nc) as tc:
        with tc.tile_pool(name="consts", bufs=1) as cpool, \
             tc.tile_pool(name="scr", bufs=2) as spool, \
             tc.tile_pool(name="ps", bufs=4, space="PSUM") as psp:
            RHS = cpool.tile([16, STREAM_CAP * 512], BF16)
            LHS = cpool.tile([16, NVAR * 128], BF16)
            TRI = cpool.tile([128, 128], BF16)
            PEN = cpool.tile([128, 128], BF16)
            QB = cpool.tile([128, QB_COLS], BF16)
            QCOL = cpool.tile([128, 8], F32)
            LIN = cpool.tile([128, 16], F32)
            ACCA = cpool.tile([128, nA], F32)
            ACCD = cpool.tile([128, max(nD, 1)], F32)
            ACCL = cpool.tile([128, 8], F32)
            R = cpool.tile([128, 4], F32)
            OUT = cpool.tile([128, 1], F32)

            nc.sync.dma_start(out=RHS[:], in_=rhs_d[:])
            nc.sync.dma_start(out=LHS[:], in_=lhs_d[:])
            nc.sync.dma_start(out=TRI[:], in_=tri_d[:])
            nc.sync.dma_start(out=PEN[:], in_=pen_d[:])
            nc.sync.dma_start(out=QCOL[:], in_=qcol_d[:])
            nc.sync.dma_start(out=LIN[:], in_=lin_d[:])
            # QB streamed per-slot so ACT-direct groups start early
            for s in range(8):
                w = ALT_CHUNKS[s] * 512
                if w:
                    nc.sync.dma_start(out=QB[:, ALT_OFFS[s]:ALT_OFFS[s] + w],
                                      in_=qb_d[:, ALT_OFFS[s]:ALT_OFFS[s] + w])

            # dep-free PE warmup while input DMAs are in flight
            DW = cpool.tile([128, 512], BF16)
            nc.gpsimd.memset(DW[:], 0.0)
            WPS = psp.tile([128, 1024], F32, tag="ps")
            for _ in range(4):
                nc.tensor.matmul(WPS[:, 0:512], DW[0:16, 0:128], DW[0:16, 0:512],
                                 start=True, stop=True)

            ia = 0
            id_ = 0
            alt_done = [False] * 8
            for gi, (s, c0, g, diag, eng) in enumerate(SCHEDULE):
                w = g * 512
                PS = psp.tile([128, 1024], F32, tag="ps")
                for b in range(g):
                    st, pos = CHUNK_MAP[(s, c0 + b)]
                    v = VMAP[(s, st)]
                    nc.tensor.matmul(PS[:, b * 512:(b + 1) * 512],
                                     LHS[:, v * 128:(v + 1) * 128],
                                     RHS[:, pos * 512:(pos + 1) * 512],
                                     start=True, stop=not (diag and b == 0))
                if diag:
                    nc.tensor.matmul(PS[:, 0:128], TRI[:], PEN[:],
                                     start=False, stop=True)
                if eng == "A":
                    SCR = spool.tile([128, 1024], F32, tag="scr")
                    nc.scalar.activation(out=SCR[:, :w], in_=PS[:, :w], func=AF.Relu,
                                         bias=0.0, scale=1.0,
                                         accum_out=ACCA[:, ia:ia + 1])
                    ia += 1
                else:
                    nc.vector.tensor_reduce(out=ACCD[:, id_:id_ + 1], in_=PS[:, :w],
                                            axis=AX.X, op=OP.add,
                                            apply_absolute_value=True)
                    id_ += 1
                # interleave ACT-direct groups after this slot's PE groups
                if not alt_done[s]:
                    last_of_slot = all(SCHEDULE[j][0] != s for j in
                                       range(gi + 1, len(SCHEDULE)))
                    if last_of_slot and ALT_CHUNKS[s] > 0:
                        wq = ALT_CHUNKS[s] * 512
                        SCR2 = spool.tile([128, 4096], F32, tag="scr2")
                        nc.scalar.activation(out=SCR2[:, :wq],
                                             in_=QB[:, ALT_OFFS[s]:ALT_OFFS[s] + wq],
                                             func=AF.Relu,
                                             bias=QCOL[:, s:s + 1], scale=1.0,
                                             accum_out=ACCA[:, ia:ia + 1])
                        ia += 1
                        alt_done[s] = True

            # linear terms: accL[:, s] = A_s * q_u + B_s
            for s in range(8):
                nc.vector.tensor_scalar(ACCL[:, s:s + 1], QCOL[:, s:s + 1],
                                        LIN[:, 2 * s:2 * s + 1],
                                        LIN[:, 2 * s + 1:2 * s + 2],
                                        OP.mult, OP.add)

            # combine: out = sum(ACCA) + 0.5*(sum(ACCD) + sum(ACCL))
            nc.vector.tensor_reduce(out=R[:, 0:1], in_=ACCA[:], axis=AX.X, op=OP.add)
            nc.vector.tensor_reduce(out=R[:, 1:2], in_=ACCD[:], axis=AX.X, op=OP.add)
            nc.vector.tensor_reduce(out=R[:, 2:3], in_=ACCL[:], axis=AX.X, op=OP.add)
            nc.vector.tensor_tensor(out=R[:, 1:2], in0=R[:, 1:2], in1=R[:, 2:3],
                                    op=OP.add)
            nc.vector.tensor_scalar(R[:, 1:2], R[:, 1:2], 0.5, None, OP.mult)
            nc.vector.tensor_tensor(out=R[:, 0:1], in0=R[:, 0:1], in1=R[:, 1:2],
                                    op=OP.add)
            nc.vector.tensor_copy(out=OUT[:], in_=R[:, 0:1])
            nc.sync.dma_start(out=out_d[:], in_=OUT[:])

    nc.finalize()
    return nc


def get_program():
    if "nc" not in _CACHE:
        _CACHE["nc"] = build_program()
    return _CACHE["nc"]

# ---------------------------------------------------------------------------
# Host side
# ---------------------------------------------------------------------------

def core_tiles(k):
    return sorted([k + 16 * m for m in range(4)] + [15 - k + 16 * m for m in range(4)])


def build_inputs(q):
    """Per-core in_maps for label-sorted preds q (np.float32 [8192])."""
    import ml_dtypes
    BF = ml_dtypes.bfloat16
    q = q.astype(np.float32)
    qb16 = q.astype(BF)
    rhs1_full = (2.0 - q).astype(np.float32).astype(BF)
    tri = np.triu(np.ones((128, 128), np.float32)).astype(BF)
    pen = np.zeros((128, 128), np.float32)
    pen[np.arange(128), np.arange(128)] = PENALTY
    pen = pen.astype(BF)

    in_maps = []
    for k in range(8):
        tiles = core_tiles(k)
        rhs = np.zeros((16, STREAM_CAP * 512), BF)
        lhs = np.zeros((16, NVAR * 128), BF)
        qbt = np.full((128, QB_COLS), PAD_VAL, np.float32).astype(BF)
        qcol = np.zeros((128, 8), np.float32)
        lin = np.zeros((128, 16), np.float32)
        for s, t in enumerate(tiles):
            real = (NBLK - t) * 128
            qcol[:, s] = qb16[t * 128:(t + 1) * 128].astype(np.float32)
            # PE-route chunks
            for c in range(PE_CHUNKS[s]):
                st, pos = CHUNK_MAP[(s, c)]
                lo = c * 512
                take = min(max(real - lo, 0), 512)
                if take > 0:
                    rhs[2 * st, pos * 512: pos * 512 + take] = np.float32(1.0)
                    rhs[2 * st + 1, pos * 512: pos * 512 + take] = \
                        rhs1_full[t * 128 + lo: t * 128 + lo + take]
                v = VMAP[(s, st)]
                lhs[2 * st, v * 128:(v + 1) * 128] = qb16[t * 128:(t + 1) * 128]
                lhs[2 * st + 1, v * 128:(v + 1) * 128] = np.float32(1.0)
            # ACT-direct chunks (tail of the window)
            for a in range(ALT_CHUNKS[s]):
                lo = (PE_CHUNKS[s] + a) * 512
                take = min(max(real - lo, 0), 512)
                col0 = ALT_OFFS[s] + a * 512
                if take > 0:
                    qbt[:, col0:col0 + take] = \
                        rhs1_full[t * 128 + lo: t * 128 + lo + take][None, :]
            # linear terms over this slot's DVE groups
            A = 0.0
            B = 0.0
            for (gs, c0, g, diag, eng) in SCHEDULE:
                if gs != s or eng != "D":
                    continue
                for b in range(g):
                    st, pos = CHUNK_MAP[(s, c0 + b)]
                    A += rhs[2 * st, pos * 512:(pos + 1) * 512].astype(np.float64).sum()
                    B += rhs[2 * st + 1, pos * 512:(pos + 1) * 512].astype(np.float64).sum()
            lin[:, 2 * s] = np.float32(A)
            lin[:, 2 * s + 1] = np.float32(B)
        in_maps.append({"rhs": rhs, "lhs": lhs, "tri": tri, "pen": pen,
                        "qb": qbt, "qcol": qcol, "linab": lin})
    return in_maps


def emulate(in_maps):
    """Numpy emulation of the device program (for offline validation)."""
    total = 0.0
    for k in range(8):
        m = in_maps[k]
        rhs = m["rhs"].astype(np.float32)
        lhs = m["lhs"].astype(np.float32)
        tri = m["tri"].astype(np.float32)
        pen = m["pen"].astype(np.float32)
        qb = m["qb"].astype(np.float32)
        qcol = m["qcol"]
        lin = m["linab"]
        accA = 0.0
        accD = 0.0
        accL = 0.0
        for (s, c0, g, diag, eng) in SCHEDULE:
            ps = np.zeros((128, g * 512), np.float64)
            for b in range(g):
                st, pos = CHUNK_MAP[(s, c0 + b)]
                v = VMAP[(s, st)]
                L = lhs[:, v * 128:(v + 1) * 128]
                Rr = rhs[:, pos * 512:(pos + 1) * 512]
                ps[:, b * 512:(b + 1) * 512] = L.T @ Rr
            if diag:
                ps[:, 0:128] += tri.T @ pen
            if eng == "A":
                accA += np.maximum(ps, 0).sum()
            else:
                accD += np.abs(ps).sum()
        for s in range(8):
            wq = ALT_CHUNKS[s] * 512
            if wq:
                t = qb[:, ALT_OFFS[s]:ALT_OFFS[s] + wq] + qcol[:, s][:, None]
                accA += np.maximum(t, 0).sum()
            accL += (lin[0, 2 * s] * qcol[:, s] + lin[0, 2 * s + 1]).sum()
        total += accA + 0.5 * (accD + accL)
    return total


def tie_correction(labels, q, order):
    ls = labels[order]
    corr = 0.0
    i = 0
    n = len(ls)
    while i < n:
        j = i + 1
        while j < n and ls[j] == ls[i]:
            j += 1
        if j - i > 1:
            for u in range(i, j):
                for v in range(u + 1, j):
                    corr += 2.0 - max(0.0, 2.0 + float(q[u]) - float(q[v]))
        i = j
    return corr


def run(inputs, trace=False):
    from concourse.bass_utils import run_bass_kernel_spmd

    preds = np.asarray(inputs["preds"], dtype=np.float32)
    labels = np.asarray(inputs["labels"], dtype=np.float32)
    order = np.argsort(labels, kind="stable")
    q = preds[order]

    nc = get_program()
    in_maps = build_inputs(q)
    res = run_bass_kernel_spmd(nc, in_maps, core_ids=list(range(8)), trace=trace)
    total = 0.0
    for c in range(8):
        total += res.results[c]["out"].astype(np.float64).sum()
    total += tie_correction(labels, q, order)
    return np.float32(total), res


def kernel(**inputs):
    out, _ = run(inputs, trace=False)
    return out



# revision 2
# speedup vs baseline: 1.0669x; 1.0669x over previous
"""Trainium2 Bass kernel v5 for nn_BatchRankingLoss (n=8192, 8 NeuronCores).

Math: with q = preds[argsort(labels)], loss = sum_{u<v} relu(2 + q_u - q_v)
(+ exact host tie correction for tied labels).

Uniform SPMD decomposition (identical program on all 8 cores):
- 64 row-tiles of 128; octile s = tiles [8s, 8s+8). Core k, slot s owns tile
  T = 8s+k: INTER pairs = rows of T x cols [(s+1)*1024, 8192) of the shared
  broadcast tile QB[128, 7168] bf16 holding (2 - q_v) replicated across
  partitions (cols 1024..8192), DMA'd tail-first in 1536-col chunks.
- INTRA-octile off-diag pairs of octile k: local tiles i<j via QBO[128, 1024]
  ((2-q) of octile k) with strips [(i+1)*128, 1024) and bias q(tile 8k+i).
- Diagonal blocks of tiles 8s+k: one K=16 stream-packed matmul (q_u+(2-q_v))
  + one repeated-penalty matmul (-1e9 on v<=u) into PSUM DIAG[128, 1024]
  (split into 512-col halves for the ISA); one bias-free relu+accum on ACT.

Reduce routes for sum relu(bias + QBx) over a piece (HW-measured rates):
- "PE": one fused DVE tensor_scalar (op0=add bias, op1=max 0, no accum;
  plain TENSOR_SCALAR opcode, 4x_2p ~0.30 ns/col) -> J bf16; PE matmul
  ones[128,1]^T @ J (~0.9 ns/col mid-pstate incl LDWEIGHTS) accumulates
  partition sums into PSROW[1, 512] across all pieces.
- "D": DVE pass1 add (4x) + pass2 max/add-reduce accum (CACHE_REDUCE, 1x):
  ~1.38 ns/col, all on DVE.
- "A": activation(Relu, bias, accum_out): ~0.92 ns/col.

Output: [128,1] partials (PE-route total folded into partition 0); host sums
partials + tie correction.
"""

import numpy as np

N = 8192
PENALTY = -240.0            # fp8e4m3 max normal; dominates |t| <= ~12

# QB chunks on the single sync ring, tail-first (multi-ring DMA measured
# SLOWER overall: concurrent streams degrade engine rates ~20%).
CHUNKS = [(6904, 8192),     # wave 0
          (5616, 6904),     # wave 1
          (4328, 5616),     # wave 2
          (3040, 4328),     # wave 3
          (1024, 3040)]     # wave 4

# HW-measured per-column ns and per-piece fixed ns (true measured rates:
# DVE fast modes mostly do NOT engage on this HW)
PE_DVE_NSC, PE_DVE_FIX = 0.42, 120.0
PE_MM_NSC, PE_MM_FIX = 1.05, 90.0       # per col / per matmul (<=512 cols)
PE_EXTRA = 2300.0                       # packed diag matmuls
D_NSC, D_FIX = 1.45, 200.0              # pass1 add + pass2 max/add CACHE
D_EXTRA = 0.0
A_NSC, A_FIX = 0.95, 700.0
A_MIN_W = 600.0                         # ACT fixed cost too big below this
ACT_EXTRA = 1283.0 + 1475.0             # table load + diag reduce
CAP = {"PE": 2048, "D": 4096, "A": 4096}
SPLIT_BOUNDS = (6904, 5616, 4328, 3040)  # piece boundaries = chunk edges

# ---------------------------------------------------------------------------
# Static schedule (identical for every core)
# ---------------------------------------------------------------------------

def _ready_idx(kind, a, b=None):
    if kind == "O":
        return 1            # QBO lands after the first QB chunk
    if b is None:
        b = a + 1
    w = 0
    for i, (lo, hi) in enumerate(CHUNKS):
        if a < hi and b > lo:
            w = max(w, i)
    return w


def _cost(e, w):
    if e == "PE":
        return (w * PE_DVE_NSC + PE_DVE_FIX,
                w * PE_MM_NSC + PE_MM_FIX * -(-w // 512))
    if e == "D":
        return (w * D_NSC + D_FIX, 0.0)
    return (0.0, 0.0)


def make_schedule():
    wins = [("I", s, (s + 1) * 1024, N) for s in range(7)]
    wins += [("O", i, (i + 1) * 128, 1024) for i in range(7)]

    load = {"D": D_EXTRA, "A": ACT_EXTRA, "M": PE_EXTRA}
    assign = []
    for kind, s, c0, c1 in sorted(wins, key=lambda it: -(it[3] - it[2])):
        w = c1 - c0
        best, bestv, bestload = None, None, None
        for e in ("PE", "D", "A"):
            if e == "A" and w < A_MIN_W:
                continue
            trial = dict(load)
            if e == "PE":
                dve, mm = _cost(e, w)
                trial["D"] += dve
                trial["M"] += mm
            elif e == "D":
                trial["D"] += w * D_NSC + D_FIX * -(-w // CAP["D"])
            else:
                trial["A"] += w * A_NSC + A_FIX * -(-w // CAP["A"])
            v = max(trial.values())
            if bestv is None or v < bestv:
                best, bestv, bestload = e, v, trial
        load = bestload
        assign.append((best, kind, s, c0, c1))

    # split at chunk boundaries (QB) and route caps; tag readiness
    pieces = []
    for e, kind, s, c0, c1 in assign:
        bounds = {c0, c1}
        if kind == "I":
            bounds |= {b for b in SPLIT_BOUNDS if c0 < b < c1}
        bounds = sorted(bounds)
        for a, b in zip(bounds, bounds[1:]):
            x = a
            while x < b:
                y = min(b, x + CAP[e])
                pieces.append((_ready_idx(kind, x, y), 0 if e == "PE" else 1,
                               e, kind, s, x, y))
                x = y
    pieces.sort()
    return [p[2:] for p in pieces], load


SCHEDULE, LOAD = make_schedule()

# ---------------------------------------------------------------------------
# Device program
# ---------------------------------------------------------------------------

_CACHE = {}


def build_program():
    import concourse.bacc as bacc
    import concourse.mybir as mybir
    from concourse.tile import TileContext

    F32 = mybir.dt.float32
    BF16 = mybir.dt.bfloat16
    FP8 = mybir.dt.float8e4
    OP = mybir.AluOpType
    AF = mybir.ActivationFunctionType
    AX = mybir.AxisListType

    n_acc = sum(1 for e, *_ in SCHEDULE if e != "PE") + 1   # + diag
    NJ = 4

    nc = bacc.Bacc(trn_type="TRN2")
    qb_d = nc.dram_tensor("qb", [128, N - 1024], BF16, kind="ExternalInput")
    qbo_d = nc.dram_tensor("qbo", [128, 1024], BF16, kind="ExternalInput")
    qc_d = nc.dram_tensor("qc", [128, 16], F32, kind="ExternalInput")
    lr16_d = nc.dram_tensor("lr16", [16, 1152], BF16, kind="ExternalInput")
    tp_d = nc.dram_tensor("tp", [128, 1152], FP8, kind="ExternalInput")
    out_d = nc.dram_tensor("out", [128, 1], F32, kind="ExternalOutput")

    with TileContext(nc) as tc:
        with tc.tile_pool(name="consts", bufs=1) as cpool, \
             tc.tile_pool(name="ps", bufs=1, space="PSUM") as psp:
            QB = cpool.tile([128, N - 1024], BF16)
            QBO = cpool.tile([128, 1024], BF16)
            QC = cpool.tile([128, 16], F32)   # cols 0-7 inter bias, 8-15 intra
            LR16 = cpool.tile([16, 1152], BF16)   # [:,0:128] lhsT, rest rhs
            TP = cpool.tile([128, 1152], FP8)     # [:,0:128] TRI, rest PEN8
            ONE = cpool.tile([128, 1], BF16)
            JB0 = cpool.tile([128, CAP["PE"]], BF16)
            JB1 = cpool.tile([128, CAP["PE"]], BF16)
            JB2 = cpool.tile([128, CAP["PE"]], BF16)
            JB3 = cpool.tile([128, CAP["PE"]], BF16)
            JB = [JB0, JB1, JB2, JB3]
            TS0 = cpool.tile([128, CAP["D"]], BF16)
            TS1 = cpool.tile([128, CAP["D"]], BF16)
            FOLD = cpool.tile([128, CAP["D"]], BF16)
            JA = cpool.tile([128, CAP["A"]], BF16)
            ACC = cpool.tile([128, n_acc], F32)
            R = cpool.tile([128, 1], F32)
            R2 = cpool.tile([1, 1], F32)

            def qb_dma(eng, ci):
                lo, hi = CHUNKS[ci]
                eng.dma_start(out=QB[:, lo - 1024:hi - 1024],
                              in_=qb_d[:, lo - 1024:hi - 1024])

            # single sync ring, tail-first; QBO after chunk 0, consts mid
            nc.sync.dma_start(out=QC[:], in_=qc_d[:])
            qb_dma(nc.sync, 0)
            nc.sync.dma_start(out=QBO[:], in_=qbo_d[:])
            qb_dma(nc.sync, 1)
            nc.sync.dma_start(out=LR16[:], in_=lr16_d[:])
            nc.sync.dma_start(out=TP[:], in_=tp_d[:])
            qb_dma(nc.sync, 2)
            qb_dma(nc.sync, 3)
            qb_dma(nc.sync, 4)
            nc.gpsimd.memset(ONE[:], 1.0)

            # packed diag blocks on PE (overlap QB stream)
            DIAG = psp.tile([128, 1024], F32, tag="diag")
            for h in range(2):
                sl = slice(h * 512, (h + 1) * 512)
                nc.tensor.matmul(DIAG[:, sl], LR16[:, 0:128],
                                 LR16[:, 128 + h * 512:128 + (h + 1) * 512],
                                 start=True, stop=False)
                nc.tensor.matmul(DIAG[:, sl], TP[:, 0:128],
                                 TP[:, 128 + h * 512:128 + (h + 1) * 512],
                                 start=False, stop=True)

            PSROW = psp.tile([1, 512], F32, tag="psrow")
            mm_total = sum(-(-(c1 - c0) // 512) for e, _, _, c0, c1 in SCHEDULE
                           if e == "PE")

            state = {"ndve": 0, "npe": 0, "imm": 0, "ia": 0}

            def emit(e, src_ap, bias_ap, w):
                if e == "PE":
                    J = JB[state["npe"] % NJ]
                    state["npe"] += 1
                    nc.vector.tensor_scalar(J[:, :w], src_ap, bias_ap,
                                            0.0, OP.add, OP.max)
                    off = 0
                    while off < w:
                        wc = min(512, w - off)
                        nc.tensor.matmul(PSROW[:, 0:wc], ONE[:],
                                         J[:, off:off + wc],
                                         start=(state["imm"] == 0),
                                         stop=(state["imm"] == mm_total - 1))
                        state["imm"] += 1
                        off += wc
                elif e == "D":
                    TS = TS0 if (state["ndve"] % 2 == 0) else TS1
                    state["ndve"] += 1
                    ia = state["ia"]
                    nc.vector.tensor_scalar(TS[:, :w], src_ap, bias_ap,
                                            None, OP.add)
                    nc.vector.tensor_scalar(FOLD[:, :w], TS[:, :w], 0.0,
                                            None, OP.max, OP.add,
                                            accum_out=ACC[:, ia:ia + 1])
                    state["ia"] += 1
                else:
                    ia = state["ia"]
                    nc.scalar.activation(out=JA[:, :w], in_=src_ap,
                                         func=AF.Relu, bias=bias_ap,
                                         scale=1.0,
                                         accum_out=ACC[:, ia:ia + 1])
                    state["ia"] += 1

            for e, kind, s, c0, c1 in SCHEDULE:
                w = c1 - c0
                if kind == "I":
                    emit(e, QB[:, c0 - 1024:c1 - 1024], QC[:, s:s + 1], w)
                else:
                    emit(e, QBO[:, c0:c1], QC[:, 8 + s:9 + s], w)

            ia = state["ia"]
            # diag reduce on ACT (PSUM, bias-free)
            nc.scalar.activation(out=JA[:, :1024], in_=DIAG[:],
                                 func=AF.Relu, bias=0.0, scale=1.0,
                                 accum_out=ACC[:, ia:ia + 1])
            ia += 1

            nc.vector.tensor_reduce(out=R[:], in_=ACC[:, :ia], axis=AX.X,
                                    op=OP.add)
            nc.vector.tensor_reduce(out=R2[:], in_=PSROW[:], axis=AX.X,
                                    op=OP.add)
            nc.vector.tensor_tensor(out=R[0:1, 0:1], in0=R[0:1, 0:1],
                                    in1=R2[:], op=OP.add)
            nc.sync.dma_start(out=out_d[:], in_=R[:])

    nc.finalize()
    return nc


def get_program():
    if "nc" not in _CACHE:
        _CACHE["nc"] = build_program()
    return _CACHE["nc"]

# ---------------------------------------------------------------------------
# Host side
# ---------------------------------------------------------------------------

def build_inputs(q):
    import ml_dtypes
    BF = ml_dtypes.bfloat16
    q = q.astype(np.float32)
    qb16f = q.astype(BF).astype(np.float32)
    r1 = (2.0 - q).astype(np.float32).astype(BF)   # bf16 (2 - q_v)
    F8 = ml_dtypes.float8_e4m3
    tri = np.triu(np.ones((128, 128), np.float32))
    pen = np.zeros((128, 128), np.float32)
    pen[np.arange(128), np.arange(128)] = PENALTY
    tp = np.concatenate([tri, np.tile(pen, (1, 8))], axis=1).astype(F8)

    qb_row = r1[1024:]
    in_maps = []
    for k in range(8):
        qb = np.broadcast_to(qb_row, (128, N - 1024)).copy()
        qbo = np.broadcast_to(r1[k * 1024:(k + 1) * 1024], (128, 1024)).copy()
        qc = np.zeros((128, 16), np.float32)
        lr16 = np.zeros((16, 1152), np.float32)
        for s in range(8):
            t = 8 * s + k
            qc[:, s] = qb16f[t * 128:(t + 1) * 128]
            lr16[2 * s, 0:128] = qb16f[t * 128:(t + 1) * 128]
            lr16[2 * s + 1, 0:128] = 1.0
            lr16[2 * s, 128 + s * 128:128 + (s + 1) * 128] = 1.0
            lr16[2 * s + 1, 128 + s * 128:128 + (s + 1) * 128] = \
                r1[t * 128:(t + 1) * 128].astype(np.float32)
            ti = 8 * k + s
            qc[:, 8 + s] = qb16f[ti * 128:(ti + 1) * 128]
        in_maps.append({"qb": qb, "qbo": qbo, "qc": qc,
                        "lr16": lr16.astype(BF), "tp": tp})
    return in_maps


def emulate(in_maps):
    import ml_dtypes
    total = 0.0
    for k in range(8):
        m = in_maps[k]
        qb = m["qb"].astype(np.float32)
        qbo = m["qbo"].astype(np.float32)
        qc = m["qc"]
        acc = 0.0
        for e, kind, s, c0, c1 in SCHEDULE:
            if kind == "I":
                blk = qb[:, c0 - 1024:c1 - 1024] + qc[:, s][:, None]
            else:
                blk = qbo[:, c0:c1] + qc[:, 8 + s][:, None]
            blk = np.maximum(blk, 0)
            if e in ("PE", "D"):
                blk = blk.astype(ml_dtypes.bfloat16)
            acc += blk.astype(np.float64).sum()
        lhs = m["lr16"].astype(np.float32)[:, 0:128]
        rhs = m["lr16"].astype(np.float32)[:, 128:]
        tri = m["tp"].astype(np.float32)[:, 0:128]
        pen8 = m["tp"].astype(np.float32)[:, 128:]
        diag = lhs.T @ rhs + np.concatenate(
            [tri.T @ pen8[:, s * 128:(s + 1) * 128] for s in range(8)], axis=1)
        acc += np.maximum(diag, 0).sum(dtype=np.float64)
        total += acc
    return total


def tie_correction(labels, q, order):
    ls = labels[order]
    corr = 0.0
    i = 0
    n = len(ls)
    while i < n:
        j = i + 1
        while j < n and ls[j] == ls[i]:
            j += 1
        if j - i > 1:
            for u in range(i, j):
                for v in range(u + 1, j):
                    corr += 2.0 - max(0.0, 2.0 + float(q[u]) - float(q[v]))
        i = j
    return corr


def run(inputs, trace=False):
    from concourse.bass_utils import run_bass_kernel_spmd

    preds = np.asarray(inputs["preds"], dtype=np.float32)
    labels = np.asarray(inputs["labels"], dtype=np.float32)
    order = np.argsort(labels, kind="stable")
    q = preds[order]

    nc = get_program()
    in_maps = build_inputs(q)
    res = run_bass_kernel_spmd(nc, in_maps, core_ids=list(range(8)), trace=trace)
    total = 0.0
    for c in range(8):
        total += res.results[c]["out"].astype(np.float64).sum()
    total += tie_correction(labels, q, order)
    return np.float32(total), res


def kernel(**inputs):
    out, _ = run(inputs, trace=False)
    return out
